# revision 17
# baseline (speedup 1.0000x reference)
"""CrossAttention Trainium2 kernel (Bass/Tile), 8-core SPMD.

Problem: q = query@Wq+bq; k = key@Wk+bk; v = value@Wv+bv;
         out = softmax(q k^T) v           (no 1/sqrt(d) scaling)
Shapes:  query [4, 2048, 1024], key/value [4, 2048, 768],
         W* [(1024|768), 1024], b* [1024], out [4, 2048, 1024] f32.

Sharding: data-parallel over (batch, query-half) -> 8 shards of 1024 query
rows. Each core redundantly projects its batch's full K/V (no collectives).

Layout: the host pre-transposes query/key/value to feature-major so the PE
contraction dim lands on partitions with plain DMAs (no on-chip input
transposes). Only the softmax-probability transpose runs on the PE.

Precision: projections + scores run the PE in float32r (rounded fp32,
1 cyc/row at N>=512; measured logit abs err ~5e-3 on sigma=32 logits);
softmax probs and V are bf16 for the final GEMM (linear error, ~2^-9).

SBUF/overlap strategy: only kT (64KB/part) and v (32KB/part) stay
SBUF-resident; qT spills to internal DRAM during stage A and streams back
per m-tile in stage D. Every stage's working set leaves >40KB/partition
free so the next stage's DMA prefetch never waits on a dying tile (WAR).
The attention m-loop is software-pipelined (AV of m-tile i runs on the PE
while softmax of m-tile i+1 occupies ACT/DVE).
"""

import os
import sys
from contextlib import ExitStack

for _p in ("/opt/trn_rl_repo", "/root/.axon_site/_ro/trn_rl_repo"):
    if os.path.isdir(_p) and _p not in sys.path:
        sys.path.append(_p)

import numpy as np

import concourse.bass as bass
import concourse.mybir as mybir
import concourse.tile as tile
from concourse import bacc
from concourse.bass import ts
from concourse.bass_utils import run_bass_kernel_spmd
from concourse.masks import make_identity

P = 128
B, LQ, LK = 4, 2048, 2048
D1, D2, H = 1024, 768, 1024
N_CORES = 8
M = (B * LQ) // N_CORES  # 1024 query rows per core

D1T, D2T, HT, MT, JT, JC = D1 // P, D2 // P, H // P, M // P, LK // P, LK // 512

F32 = mybir.dt.float32
F32R = mybir.dt.float32r
BF16 = mybir.dt.bfloat16
AX = mybir.AxisListType.X
AF = mybir.ActivationFunctionType
ALU = mybir.AluOpType

_CACHE = {}
LAST_RESULTS = None  # BassKernelResults of the most recent run (for test harness)


def _build_bass():
    nc = bacc.Bacc("TRN2", target_bir_lowering=False, debug=False,
                   num_devices=N_CORES)

    # All big operands arrive feature-major (pre-transposed on the host).
    xqt = nc.dram_tensor("xqt", [D1, M], F32R, kind="ExternalInput")
    kyt = nc.dram_tensor("kyt", [D2, LK // 2], F32R, kind="ExternalInput")
    vvt = nc.dram_tensor("vvt", [D2, LK // 2], BF16, kind="ExternalInput")
    wq = nc.dram_tensor("wq", [D1, H], F32R, kind="ExternalInput")
    wk = nc.dram_tensor("wk", [D2, H], F32R, kind="ExternalInput")
    wv = nc.dram_tensor("wv", [D2, H], BF16, kind="ExternalInput")
    bqd = nc.dram_tensor("bq", [H], F32, kind="ExternalInput")
    bkd = nc.dram_tensor("bk", [H], F32, kind="ExternalInput")
    bvd = nc.dram_tensor("bv", [H], F32, kind="ExternalInput")
    out = nc.dram_tensor("out", [M, H], F32, kind="ExternalOutput")
    # pair-AllGather staging: kT in two 512-col chunks, v in one shot
    kb_in = nc.dram_tensor("kb_in", [2, P, HT, 512], F32R)
    kb_out = nc.dram_tensor("kb_out", [2, 2, P, HT, 512], F32R)
    vb_in = nc.dram_tensor("vb_in", [P, JT // 2, H], BF16)
    vb_out = nc.dram_tensor("vb_out", [2, P, JT // 2, H], BF16)

    wq_t = wq.rearrange("(t p) h -> p t h", p=P)
    wk_t = wk.rearrange("(t p) h -> p t h", p=P)
    wv_t = wv.rearrange("(t p) h -> p t h", p=P)
    xqt_t = xqt.rearrange("(t p) m -> p t m", p=P)
    kyt_t = kyt.rearrange("(t p) j -> p t j", p=P)
    vvt_t = vvt.rearrange("(t p) j -> p t j", p=P)

    with tile.TileContext(nc) as tc, ExitStack() as top:
        const = top.enter_context(tc.tile_pool(name="const", bufs=1))
        bias2 = const.tile([P, 2, HT], F32)
        nc.sync.dma_start(bias2[:, 0, :], bqd.rearrange("(t p) -> p t", p=P))
        nc.sync.dma_start(bias2[:, 1, :], bkd.rearrange("(t p) -> p t", p=P))
        bqt = bias2[:, 0, :]
        bkt = bias2[:, 1, :]

        # Shared PSUM pools: 2 transpose banks + 3 accumulation banks.
        tpool = top.enter_context(tc.tile_pool(name="tpool", bufs=2,
                                               space="PSUM"))
        pps = top.enter_context(tc.tile_pool(name="pps", bufs=3, space="PSUM"))

        # Residents: qT [H, M] + kT [H, LK] f32r.
        respool = top.enter_context(tc.tile_pool(name="res", bufs=1))
        qT = respool.tile([P, HT, M], F32R)
        kT = respool.tile([P, HT, LK], F32R)

        # Stage-B pools live on the RIGHT SBUF stack and are opened before
        # stage A so their prefetch DMAs don't inherit a WAR dependency on
        # stage A's buffers (the left stack rewinds into A's range).
        esB = top.enter_context(ExitStack())
        sb1 = esB.enter_context(tc.tile_pool(name="sb1", bufs=1, side="right"))
        sb3 = esB.enter_context(tc.tile_pool(name="sb3", bufs=2, side="right"))
        wk_h0 = sb1.tile([P, D2T // 2, H], F32R)

        # ---- Stage A: qT[h, m] = Wq^T @ X^T + bq ----
        # One need-ordered queue (sync): wq first half, X^T, wq second
        # half, then B's weights — so the PE is fed from ~7us on.
        with tc.tile_pool(name="sa1", bufs=1, side="left") as sa1:
            wqs = sa1.tile([P, D1T, H], F32R)
            xTs = sa1.tile([P, D1T, M], F32R)
            nc.sync.dma_start(wqs[:, :, 0:512], wq_t[:, :, 0:512])
            for dt in range(D1T):
                nc.sync.dma_start(xTs[:, dt, :], xqt_t[:, dt, :])
            nc.sync.dma_start(wqs[:, :, 512:1024], wq_t[:, :, 512:1024])
            nc.sync.dma_start(wk_h0[:], wk_t[:, 0:D2T // 2, :])
            for ht in range(HT):
                for mc in range(M // 512):
                    psq = pps.tile([P, 512], F32, tag="acc")
                    for dt in range(D1T):
                        nc.tensor.matmul(psq[:], wqs[:, dt, ts(ht, P)],
                                         xTs[:, dt, ts(mc, 512)],
                                         start=(dt == 0), stop=(dt == D1T - 1))
                    nc.scalar.activation(qT[:, ht, ts(mc, 512)], psq[:],
                                         AF.Identity, bias=bqt[:, ht:ht + 1],
                                         scale=1.0)

        # Stage-C pools (left stack, reusing stage A's range) open now so
        # wvs/vTc prefetch runs during stage B.
        esC = top.enter_context(ExitStack())
        vpool = top.enter_context(tc.tile_pool(name="vres", bufs=1,
                                               side="left"))
        vsb = vpool.tile([P, JT, H], BF16)
        sc1 = esC.enter_context(tc.tile_pool(name="sc1", bufs=1, side="left"))
        sc3 = esC.enter_context(tc.tile_pool(name="sc3", bufs=2, side="left"))
        wvs = sc1.tile([P, D2T, H], BF16)

        # ---- Stage B: kT own j-half (2 chunks), pair-AllGather per chunk ----
        with tc.tile_pool(name="sbh", bufs=1, side="left") as sbh:
            # Second half of Wk allocates (and loads) as soon as A frees.
            wk_h1 = sbh.tile([P, D2T // 2, H], F32R)
            nc.sync.dma_start(wk_h1[:], wk_t[:, D2T // 2:D2T, :])
            for jc in range(2):
                yTc = sb3.tile([P, D2T, 512], F32R, tag="yTc")
                nc.sync.dma_start(yTc[:], kyt_t[:, :, ts(jc, 512)])
                if jc == 1:
                    nc.sync.dma_start(wvs[:], wv_t[:])
                for ht in range(HT):
                    psk = pps.tile([P, 512], F32, tag="acc")
                    for dt in range(D2T):
                        wsl = (wk_h0 if dt < D2T // 2 else wk_h1)
                        nc.tensor.matmul(psk[:],
                                         wsl[:, dt % (D2T // 2), ts(ht, P)],
                                         yTc[:, dt, :],
                                         start=(dt == 0), stop=(dt == D2T - 1))
                    nc.scalar.activation(kT[:, ht, ts(jc, 512)], psk[:],
                                         AF.Identity, bias=bkt[:, ht:ht + 1],
                                         scale=1.0)
                nc.scalar.dma_start(kb_in[jc], kT[:, :, ts(jc, 512)])
                nc.gpsimd.collective_compute(
                    "AllGather", ALU.bypass,
                    replica_groups=[[0, 1], [2, 3], [4, 5], [6, 7]],
                    ins=[kb_in[jc]], outs=[kb_out[jc]])
                for g in range(2):
                    nc.scalar.dma_start(
                        kT[:, :, bass.ds(g * 1024 + jc * 512, 512)],
                        kb_out[jc, g].rearrange("p t j -> p t j"))

        # Stage-B pools released (right stack); stage-D pools take their
        # place on the right so softmax buffers prefetch during stage C.
        esB.close()
        esD = top.enter_context(ExitStack())
        sd2 = esD.enter_context(tc.tile_pool(name="sd2", bufs=2, side="right"))
        sd3 = esD.enter_context(tc.tile_pool(name="sd3", bufs=2, side="right"))
        sdc = esD.enter_context(tc.tile_pool(name="sdc", bufs=1, side="right"))
        stat = esD.enter_context(tc.tile_pool(name="stat", bufs=3,
                                              side="right"))
        identb = sdc.tile([P, P], BF16)
        make_identity(nc, identb[:])
        bv_full = sdc.tile([P, H], F32)
        nc.sync.dma_start(bv_full[:], bvd[None, :].to_broadcast([P, H]))

        # ---- Stage C: v[j, h] = Vin^T-blocks @ Wv (bv folded in at the end) ----
        if True:
            for jc in range(2):
                vTc = sc3.tile([P, D2T, 512], BF16, tag="vTc")
                nc.sync.dma_start(vTc[:], vvt_t[:, :, ts(jc, 512)])
                for jt4 in range(4):
                    jt = jc * 4 + jt4
                    for hc in range(H // 512):
                        psv = pps.tile([P, 512], F32, tag="acc")
                        for dt in range(D2T):
                            nc.tensor.matmul(psv[:], vTc[:, dt, ts(jt4, P)],
                                             wvs[:, dt, ts(hc, 512)],
                                             start=(dt == 0),
                                             stop=(dt == D2T - 1))
                        nc.vector.tensor_copy(vsb[:, jt, ts(hc, 512)], psv[:])
            nc.scalar.dma_start(vb_in[:], vsb[:, 0:JT // 2, :])
            nc.gpsimd.collective_compute(
                "AllGather", ALU.bypass,
                replica_groups=[[0, 1], [2, 3], [4, 5], [6, 7]],
                ins=[vb_in[:]], outs=[vb_out[:]])
            for g in range(2):
                nc.scalar.dma_start(vsb[:, bass.ds(g * (JT // 2), JT // 2), :],
                                    vb_out[g])

        # ---- Stage D: per m-tile scores -> softmax -> (probs^T) @ v ----
        # Software-pipelined: AV of m-tile i is emitted after the softmax/
        # transpose of m-tile i+1 has been set in motion.
        esC.close()
        if True:
            def scores_softmax(mt):
                ssb = sd2.tile([P, JC, 512], F32, tag="ssb")
                mx4 = stat.tile([P, JC], F32, tag="mx4")
                for jc in range(JC):
                    pss = pps.tile([P, 512], F32, tag="acc")
                    for ht in range(HT):
                        nc.tensor.matmul(pss[:], qT[:, ht, ts(mt, P)],
                                         kT[:, ht, ts(jc, 512)],
                                         start=(ht == 0), stop=(ht == HT - 1))
                    nc.vector.tensor_copy(ssb[:, jc, :], pss[:])
                    nc.vector.reduce_max(mx4[:, jc:jc + 1], pss[:], axis=AX)
                negmax = stat.tile([P, 1], F32, tag="negmax")
                nc.vector.reduce_max(negmax[:], mx4[:], axis=AX, negate=True)
                wsb = sd2.tile([P, JC, 512], BF16, tag="wsb")
                sm4 = stat.tile([P, JC], F32, tag="sm4")
                for jc in range(JC):
                    nc.scalar.activation(wsb[:, jc, :], ssb[:, jc, :], AF.Exp,
                                         bias=negmax[:, 0:1], scale=1.0,
                                         accum_out=sm4[:, jc:jc + 1])
                ssum = stat.tile([P, 1], F32, tag="ssum")
                nc.vector.reduce_sum(ssum[:], sm4[:], axis=AX)
                rinv = stat.tile([P, 1], F32, tag="rinv")
                nc.vector.reciprocal(rinv[:], ssum[:])
                wT = sd3.tile([P, JT, P], BF16, tag="wT")
                for a in (0, 4, 8, 12):
                    pst = tpool.tile([P, 512], BF16, tag="tpb")
                    for g in range(4):
                        jt = a + g
                        nc.tensor.transpose(pst[:, ts(g, P)],
                                            wsb[:, jt // 4, ts(jt % 4, P)],
                                            identb[:])
                    nc.vector.tensor_copy(
                        wT[:, a:a + 4, :],
                        pst[:].rearrange("p (a b) -> p a b", a=4))
                return wT, rinv

            def av(mt, wT, rinv):
                osb = sd2.tile([P, H], F32, tag="osb")
                for hc in range(H // 512):
                    psa = pps.tile([P, 512], F32, tag="acc")
                    for jt in range(JT):
                        nc.tensor.matmul(psa[:], wT[:, jt, :],
                                         vsb[:, jt, ts(hc, 512)],
                                         start=(jt == 0), stop=(jt == JT - 1))
                    nc.scalar.activation(osb[:, ts(hc, 512)], psa[:], AF.Copy,
                                         scale=rinv[:, 0:1])
                nc.vector.tensor_tensor(osb[:], osb[:], bv_full[:], ALU.add)
                nc.sync.dma_start(out[ts(mt, P), :], osb[:])

            prev = None
            for mt in range(MT):
                cur = scores_softmax(mt)
                if prev is not None:
                    av(prev[0], prev[1], prev[2])
                prev = (mt,) + cur
            av(prev[0], prev[1], prev[2])

    nc.compile()
    return nc


def _get_nc():
    if "nc" not in _CACHE:
        _CACHE["nc"] = _build_bass()
    return _CACHE["nc"]


def kernel(query, key, value, Wq, bq, Wk, bk, Wv, bv):
    global LAST_RESULTS
    nc = _get_nc()

    def f(a):
        return np.ascontiguousarray(np.asarray(a, dtype=np.float32))

    query, key, value = f(query), f(key), f(value)
    Wq, bq, Wk, bk, Wv, bv = f(Wq), f(bq), f(Wk), f(bk), f(Wv), f(bv)

    in_maps = []
    half = LQ // 2
    import ml_dtypes
    keyT = [np.ascontiguousarray(key[b].T) for b in range(B)]
    valT = [np.ascontiguousarray(value[b].T.astype(ml_dtypes.bfloat16))
            for b in range(B)]
    Wv = Wv.astype(ml_dtypes.bfloat16)
    halfk = LK // 2
    for c in range(N_CORES):
        b, h = divmod(c, 2)
        in_maps.append({
            "xqt": np.ascontiguousarray(query[b, h * half:(h + 1) * half, :].T),
            "kyt": np.ascontiguousarray(keyT[b][:, h * halfk:(h + 1) * halfk]),
            "vvt": np.ascontiguousarray(valT[b][:, h * halfk:(h + 1) * halfk]),
            "wq": Wq, "wk": Wk, "wv": Wv,
            "bq": bq, "bk": bk, "bv": bv,
        })

    res = run_bass_kernel_spmd(nc, in_maps, core_ids=list(range(N_CORES)))
    LAST_RESULTS = res

    out = np.empty((B, LQ, H), dtype=np.float32)
    for c in range(N_CORES):
        b, h = divmod(c, 2)
        out[b, h * half:(h + 1) * half, :] = res.results[c]["out"]
    return out


# revision 19
# speedup vs baseline: 1.2801x; 1.2801x over previous
"""CrossAttention Trainium2 kernel (Bass/Tile), 8-core SPMD.

Problem: q = query@Wq+bq; k = key@Wk+bk; v = value@Wv+bv;
         out = softmax(q k^T) v           (no 1/sqrt(d) scaling)
Shapes:  query [4, 2048, 1024], key/value [4, 2048, 768],
         W* [(1024|768), 1024], b* [1024], out [4, 2048, 1024] f32.

Sharding: data-parallel over (batch, query-half) -> 8 shards of 1024 query
rows. Each core redundantly projects its batch's full K/V (no collectives).

Layout: the host pre-transposes query/key/value to feature-major so the PE
contraction dim lands on partitions with plain DMAs (no on-chip input
transposes). Only the softmax-probability transpose runs on the PE.

Precision: projections + scores run the PE in float32r (rounded fp32,
1 cyc/row at N>=512; measured logit abs err ~5e-3 on sigma=32 logits);
softmax probs and V are bf16 for the final GEMM (linear error, ~2^-9).

SBUF/overlap strategy: only kT (64KB/part) and v (32KB/part) stay
SBUF-resident; qT spills to internal DRAM during stage A and streams back
per m-tile in stage D. Every stage's working set leaves >40KB/partition
free so the next stage's DMA prefetch never waits on a dying tile (WAR).
The attention m-loop is software-pipelined (AV of m-tile i runs on the PE
while softmax of m-tile i+1 occupies ACT/DVE).
"""

import os
import sys
from contextlib import ExitStack

for _p in ("/opt/trn_rl_repo", "/root/.axon_site/_ro/trn_rl_repo"):
    if os.path.isdir(_p) and _p not in sys.path:
        sys.path.append(_p)

import numpy as np

import concourse.bass as bass
import concourse.mybir as mybir
import concourse.tile as tile
from concourse import bacc
from concourse.bass import ts
from concourse.bass_utils import run_bass_kernel_spmd
from concourse.masks import make_identity

P = 128
B, LQ, LK = 4, 2048, 2048
D1, D2, H = 1024, 768, 1024
N_CORES = 8
M = (B * LQ) // N_CORES  # 1024 query rows per core

D1T, D2T, HT, MT, JT, JC = D1 // P, D2 // P, H // P, M // P, LK // P, LK // 512

F32 = mybir.dt.float32
F32R = mybir.dt.float32r
BF16 = mybir.dt.bfloat16
AX = mybir.AxisListType.X
AF = mybir.ActivationFunctionType
ALU = mybir.AluOpType

_CACHE = {}
LAST_RESULTS = None  # BassKernelResults of the most recent run (for test harness)


def _build_bass():
    nc = bacc.Bacc("TRN2", target_bir_lowering=False, debug=False,
                   num_devices=N_CORES)

    # All big operands arrive feature-major (pre-transposed on the host).
    xqt = nc.dram_tensor("xqt", [D1, M], F32R, kind="ExternalInput")
    kyt = nc.dram_tensor("kyt", [D2, LK], F32R, kind="ExternalInput")
    vvt = nc.dram_tensor("vvt", [D2, LK], BF16, kind="ExternalInput")
    wq = nc.dram_tensor("wq", [D1, H], F32R, kind="ExternalInput")
    wk = nc.dram_tensor("wk", [D2, H], F32R, kind="ExternalInput")
    wv = nc.dram_tensor("wv", [D2, H], BF16, kind="ExternalInput")
    bqd = nc.dram_tensor("bq", [H], F32, kind="ExternalInput")
    bkd = nc.dram_tensor("bk", [H], F32, kind="ExternalInput")
    bvd = nc.dram_tensor("bv", [H], F32, kind="ExternalInput")
    out = nc.dram_tensor("out", [M, H], F32, kind="ExternalOutput")

    wq_t = wq.rearrange("(t p) h -> p t h", p=P)
    wk_t = wk.rearrange("(t p) h -> p t h", p=P)
    wv_t = wv.rearrange("(t p) h -> p t h", p=P)
    xqt_t = xqt.rearrange("(t p) m -> p t m", p=P)
    kyt_t = kyt.rearrange("(t p) j -> p t j", p=P)
    vvt_t = vvt.rearrange("(t p) j -> p t j", p=P)

    with tile.TileContext(nc) as tc, ExitStack() as top:
        const = top.enter_context(tc.tile_pool(name="const", bufs=1))
        bias2 = const.tile([P, 2, HT], F32)
        nc.sync.dma_start(bias2[:, 0, :], bqd.rearrange("(t p) -> p t", p=P))
        nc.sync.dma_start(bias2[:, 1, :], bkd.rearrange("(t p) -> p t", p=P))
        bqt = bias2[:, 0, :]
        bkt = bias2[:, 1, :]

        # Shared PSUM accumulation pool (scores / projections / AV).
        pps = top.enter_context(tc.tile_pool(name="pps", bufs=5, space="PSUM"))

        # Residents: qT [H, M] + kT [H, LK] f32r.
        respool = top.enter_context(tc.tile_pool(name="res", bufs=1))
        qT = respool.tile([P, HT, M], F32R)
        kT = respool.tile([P, HT, LK], F32R)

        # Stage-B pools live on the RIGHT SBUF stack and are opened before
        # stage A so their prefetch DMAs don't inherit a WAR dependency on
        # stage A's buffers (the left stack rewinds into A's range).
        esB = top.enter_context(ExitStack())
        sb1 = esB.enter_context(tc.tile_pool(name="sb1", bufs=1, side="right"))
        sb3 = esB.enter_context(tc.tile_pool(name="sb3", bufs=2, side="right"))
        wk_h0 = sb1.tile([P, D2T // 2, H], F32R)

        # ---- Stage A: qT[h, m] = Wq^T @ X^T + bq ----
        # One need-ordered queue (sync): wq first half, X^T, wq second
        # half, then B's weights — so the PE is fed from ~7us on.
        with tc.tile_pool(name="sa1", bufs=1, side="left") as sa1:
            wqs = sa1.tile([P, D1T, H], F32R)
            xTs = sa1.tile([P, D1T, M], F32R)
            nc.sync.dma_start(wqs[:, :, 0:512], wq_t[:, :, 0:512])
            for dt in range(D1T):
                nc.sync.dma_start(xTs[:, dt, :], xqt_t[:, dt, :])
            nc.sync.dma_start(wqs[:, :, 512:1024], wq_t[:, :, 512:1024])
            nc.sync.dma_start(wk_h0[:], wk_t[:, 0:D2T // 2, :])
            for ht in range(HT):
                for mc in range(M // 512):
                    psq = pps.tile([P, 512], F32, tag="acc")
                    for dt in range(D1T):
                        nc.tensor.matmul(psq[:], wqs[:, dt, ts(ht, P)],
                                         xTs[:, dt, ts(mc, 512)],
                                         start=(dt == 0), stop=(dt == D1T - 1))
                    nc.scalar.activation(qT[:, ht, ts(mc, 512)], psq[:],
                                         AF.Identity, bias=bqt[:, ht:ht + 1],
                                         scale=1.0)

        # Stage-C pools (left stack, reusing stage A's range) open now so
        # wvs/vTc prefetch runs during stage B.
        esC = top.enter_context(ExitStack())
        vpool = top.enter_context(tc.tile_pool(name="vres", bufs=1,
                                               side="left"))
        vsb = vpool.tile([P, JT, H], BF16)
        sc1 = esC.enter_context(tc.tile_pool(name="sc1", bufs=1, side="left"))
        sc3 = esC.enter_context(tc.tile_pool(name="sc3", bufs=2, side="left"))
        wvs = sc1.tile([P, D2T, H], BF16)

        # ---- Stage B: kT[h, j] = Wk^T @ Y^T + bk ----
        with tc.tile_pool(name="sbh", bufs=1, side="left") as sbh:
            # Second half of Wk allocates (and loads) as soon as A frees.
            wk_h1 = sbh.tile([P, D2T // 2, H], F32R)
            nc.sync.dma_start(wk_h1[:], wk_t[:, D2T // 2:D2T, :])
            for jc in range(JC):
                yTc = sb3.tile([P, D2T, 512], F32R, tag="yTc")
                nc.sync.dma_start(yTc[:], kyt_t[:, :, ts(jc, 512)])
                if jc == 1:
                    nc.sync.dma_start(wvs[:], wv_t[:])
                for ht in range(HT):
                    psk = pps.tile([P, 512], F32, tag="acc")
                    for dt in range(D2T):
                        wsl = (wk_h0 if dt < D2T // 2 else wk_h1)
                        nc.tensor.matmul(psk[:],
                                         wsl[:, dt % (D2T // 2), ts(ht, P)],
                                         yTc[:, dt, :],
                                         start=(dt == 0), stop=(dt == D2T - 1))
                    nc.scalar.activation(kT[:, ht, ts(jc, 512)], psk[:],
                                         AF.Identity, bias=bkt[:, ht:ht + 1],
                                         scale=1.0)

        # Stage-B pools released (right stack); stage-D pools take their
        # place on the right so softmax buffers prefetch during stage C.
        esB.close()
        esD = top.enter_context(ExitStack())
        sd2 = esD.enter_context(tc.tile_pool(name="sd2", bufs=2, side="right"))
        sd3 = esD.enter_context(tc.tile_pool(name="sd3", bufs=2, side="right"))
        sdc = esD.enter_context(tc.tile_pool(name="sdc", bufs=1, side="right"))
        stat = esD.enter_context(tc.tile_pool(name="stat", bufs=3,
                                              side="right"))
        bv_full = sdc.tile([P, H], F32)
        nc.sync.dma_start(bv_full[:], bvd[None, :].to_broadcast([P, H]))

        # ---- Stage C: v[j, h] = Vin^T-blocks @ Wv (bv folded in at the end) ----
        if True:
            for jc in range(JC):
                vTc = sc3.tile([P, D2T, 512], BF16, tag="vTc")
                nc.sync.dma_start(vTc[:], vvt_t[:, :, ts(jc, 512)])
                for jt4 in range(4):
                    jt = jc * 4 + jt4
                    for hc in range(H // 512):
                        psv = pps.tile([P, 512], F32, tag="acc")
                        for dt in range(D2T):
                            nc.tensor.matmul(psv[:], vTc[:, dt, ts(jt4, P)],
                                             wvs[:, dt, ts(hc, 512)],
                                             start=(dt == 0),
                                             stop=(dt == D2T - 1))
                        nc.vector.tensor_copy(vsb[:, jt, ts(hc, 512)], psv[:])

        # ---- Stage D: per m-tile scores -> softmax -> (probs^T) @ v ----
        # Software-pipelined: AV of m-tile i is emitted after the softmax/
        # transpose of m-tile i+1 has been set in motion.
        esC.close()
        if True:
            def scores_softmax(mt):
                ssb = sd2.tile([P, JC, 512], F32, tag="ssb")
                mx4 = stat.tile([P, JC], F32, tag="mx4")
                for jc in range(JC):
                    pss = pps.tile([P, 512], F32, tag="acc")
                    for ht in range(HT):
                        nc.tensor.matmul(pss[:], qT[:, ht, ts(mt, P)],
                                         kT[:, ht, ts(jc, 512)],
                                         start=(ht == 0), stop=(ht == HT - 1))
                    nc.vector.tensor_copy(ssb[:, jc, :], pss[:])
                    nc.vector.reduce_max(mx4[:, jc:jc + 1], pss[:], axis=AX)
                negmax = stat.tile([P, 1], F32, tag="negmax")
                nc.vector.reduce_max(negmax[:], mx4[:], axis=AX, negate=True)
                wsb = sd2.tile([P, JC, 512], BF16, tag="wsb")
                sm4 = stat.tile([P, JC], F32, tag="sm4")
                for jc in range(JC):
                    nc.scalar.activation(wsb[:, jc, :], ssb[:, jc, :], AF.Exp,
                                         bias=negmax[:, 0:1], scale=1.0,
                                         accum_out=sm4[:, jc:jc + 1])
                ssum = stat.tile([P, 1], F32, tag="ssum")
                nc.vector.reduce_sum(ssum[:], sm4[:], axis=AX)
                rinv = stat.tile([P, 1], F32, tag="rinv")
                nc.vector.reciprocal(rinv[:], ssum[:])
                wT = sd3.tile([P, JT, P], BF16, tag="wT")
                nc.scalar.dma_start_transpose(
                    wT[:], wsb[:].rearrange("p a b -> p (a b)"))
                return wT, rinv

            def av(mt, wT, rinv):
                osb = sd2.tile([P, H], F32, tag="osb")
                for hc in range(H // 512):
                    psa = pps.tile([P, 512], F32, tag="acc")
                    for jt in range(JT):
                        nc.tensor.matmul(psa[:], wT[:, jt, :],
                                         vsb[:, jt, ts(hc, 512)],
                                         start=(jt == 0), stop=(jt == JT - 1))
                    nc.scalar.activation(osb[:, ts(hc, 512)], psa[:], AF.Copy,
                                         scale=rinv[:, 0:1])
                nc.vector.tensor_tensor(osb[:], osb[:], bv_full[:], ALU.add)
                nc.sync.dma_start(out[ts(mt, P), :], osb[:])

            prev = None
            for mt in range(MT):
                cur = scores_softmax(mt)
                if prev is not None:
                    av(prev[0], prev[1], prev[2])
                prev = (mt,) + cur
            av(prev[0], prev[1], prev[2])

    nc.compile()
    return nc


def _get_nc():
    if "nc" not in _CACHE:
        _CACHE["nc"] = _build_bass()
    return _CACHE["nc"]


def kernel(query, key, value, Wq, bq, Wk, bk, Wv, bv):
    global LAST_RESULTS
    nc = _get_nc()

    def f(a):
        return np.ascontiguousarray(np.asarray(a, dtype=np.float32))

    query, key, value = f(query), f(key), f(value)
    Wq, bq, Wk, bk, Wv, bv = f(Wq), f(bq), f(Wk), f(bk), f(Wv), f(bv)

    in_maps = []
    half = LQ // 2
    import ml_dtypes
    keyT = [np.ascontiguousarray(key[b].T) for b in range(B)]
    valT = [np.ascontiguousarray(value[b].T.astype(ml_dtypes.bfloat16))
            for b in range(B)]
    Wv = Wv.astype(ml_dtypes.bfloat16)
    for c in range(N_CORES):
        b, h = divmod(c, 2)
        in_maps.append({
            "xqt": np.ascontiguousarray(query[b, h * half:(h + 1) * half, :].T),
            "kyt": keyT[b],
            "vvt": valT[b],
            "wq": Wq, "wk": Wk, "wv": Wv,
            "bq": bq, "bk": bk, "bv": bv,
        })

    res = run_bass_kernel_spmd(nc, in_maps, core_ids=list(range(N_CORES)))
    LAST_RESULTS = res

    out = np.empty((B, LQ, H), dtype=np.float32)
    for c in range(N_CORES):
        b, h = divmod(c, 2)
        out[b, h * half:(h + 1) * half, :] = res.results[c]["out"]
    return out
